# revision 1
# baseline (speedup 1.0000x reference)
"""Adaptive per-pixel LoG 9x9 convolution on 8 TRN2 NeuronCores.

out[b,c,y,x] = sum_{dy,dx in [-4,4]} xpad[b,c,y+dy,x+dx] * K(dx^2+dy^2; p)
K depends on the offset only through r2 = dx^2+dy^2 (15 distinct values)
-> exact rank-15 decomposition  out[c,p] = sum_v w_v[p] * S_v[c,p]  where
S_v are fixed ring-sum convolutions (shared shifted adds) and
w_v = (base - r2_v*B2) * exp(-r2_v * inv2s2); base/B2/inv2s2 are smooth
per-pixel scalar fields from the focus-of-attention distance (host-prepared,
the exp lives on the scalar engine).

Sharding: 8 cores = 4 batches x 2 row-halves. Partition p = 16x16 output tile
+ 4px halo (24x24 window, 3 channels); all taps are free-dim AP offsets; host
bakes the window layout so DMAs are contiguous.

Perf: bf16 on the DVE (2x_1p; xp1 = 1-col-shifted copy keeps taps 4B-aligned).
HW cost is ~142cyc/op + 0.5cyc/elem -> work is batched into ~29 wide ops:
4 row-pair ops cover 16 ring partial sums, one op multiplies all 14 weight
planes by the exps, one mega-op forms 10 of the 15 products (zero-stride
channel broadcast), 5-op tree reduction. Critical input DMAs ride the two
HWDGE queues (sync+scalar, xp split 74/54 to equalize queue-rate skew); slow SWDGE (gpsimd) carries
xp1+inv; D0 ops cover the xp1 latency. Output ships bf16 on three queues.
"""

import math

import numpy as np

B, C, H, W = 4, 3, 256, 256
PAD = 4
SIGMA_MIN, SIGMA_MAX = 0.5, 10.0
N_CORES = 8

S_ROWS = 16
S_COLS = 16
N_STRIPS = 8
N_BLOCKS = 16
IN_R = 24
IN_C = 24
IN_C1 = 22

R2_VALUES = sorted({dx * dx + dy * dy for dx in range(-4, 5) for dy in range(-4, 5)})
assert len(R2_VALUES) == 15
NV = 15
# ring order: slot 0 = r2=0; S-resident rings 1..10; D-diagonal rings 11..14
V_ORD = [1, 4, 9, 16, 5, 10, 17, 13, 20, 25, 2, 8, 18, 32]


def _build_program(nc, bass, mybir):
    f32 = mybir.dt.float32
    bf16 = mybir.dt.bfloat16
    Alu = mybir.AluOpType
    Act = mybir.ActivationFunctionType

    xp_d = nc.declare_dram_parameter("xp", [128, C, IN_R, IN_C], bf16, isOutput=False)
    xp1_d = nc.declare_dram_parameter("xp1", [128, C, IN_R, IN_C1], bf16, isOutput=False)
    wv_d = nc.declare_dram_parameter("wv", [128, NV, S_ROWS, S_COLS], bf16, isOutput=False)
    inv_d = nc.declare_dram_parameter("inv", [128, S_ROWS, S_COLS], f32, isOutput=False)
    out_d = nc.declare_dram_parameter("out", [128, C, S_ROWS, S_COLS], bf16, isOutput=True)

    with (
        nc.Block() as block,
        nc.semaphore("xa_sem") as xa_sem,
        nc.semaphore("x1a_sem") as x1a_sem,
        nc.semaphore("xb_sem") as xb_sem,
        nc.semaphore("inv_sem") as inv_sem,
        nc.semaphore("wv_sem") as wv_sem,
        nc.semaphore("act_sem") as act_sem,
        nc.semaphore("dve_sem") as dve_sem,
        nc.semaphore("od_sem") as od_sem,
        nc.sbuf_tensor("s_xp", [128, C, IN_R, IN_C], bf16) as xp,
        nc.sbuf_tensor("s_xp1", [128, C, IN_R, IN_C1], bf16) as xp1,
        nc.sbuf_tensor("s_wv", [128, NV, S_ROWS, S_COLS], bf16) as Wv,
        nc.sbuf_tensor("s_inv", [128, S_ROWS, S_COLS], f32) as inv,
        nc.sbuf_tensor("U", [128, 4, C, IN_R, S_COLS], bf16) as U,
        nc.sbuf_tensor("D", [128, 4, 4, C, S_ROWS, S_COLS], bf16) as D,
        nc.sbuf_tensor("D0", [128, 4, C, S_ROWS, S_COLS], bf16) as D0,
        nc.sbuf_tensor("S", [128, 11, C, S_ROWS, S_COLS], bf16) as S,
        nc.sbuf_tensor("E", [128, 14, S_ROWS, S_COLS], bf16) as E,
        nc.sbuf_tensor("P", [128, NV, C, S_ROWS, S_COLS], bf16) as P,
    ):
        @block.sync
        def _(sync):
            sync.dma_start(out=xp[0:74], in_=xp_d[0:74]).then_inc(xa_sem, 16)
            sync.dma_start(out=Wv[:], in_=wv_d[:]).then_inc(wv_sem, 16)
            sync.wait_ge(dve_sem, 1)
            sync.dma_start(out=out_d[:, 0], in_=P[:, 0, 0]).then_inc(od_sem, 16)
            sync.wait_ge(od_sem, 48)

        @block.gpsimd
        def _(gpsimd):
            gpsimd.dma_start(out=xp1[:], in_=xp1_d[:]).then_inc(x1a_sem, 16)
            gpsimd.dma_start(out=inv[:], in_=inv_d[:]).then_inc(inv_sem, 16)
            gpsimd.wait_ge(dve_sem, 3)
            gpsimd.dma_start(out=out_d[:, 2], in_=P[:, 0, 2]).then_inc(od_sem, 16)
            gpsimd.wait_ge(od_sem, 48)

        @block.scalar
        def _(scalar):
            scalar.dma_start(out=xp[74:128], in_=xp_d[74:128]).then_inc(xb_sem, 16)
            scalar.wait_ge(inv_sem, 16)
            for i, v in enumerate(V_ORD):
                scalar.activation(
                    E[:, i], inv[:], Act.Exp, bias=0.0, scale=float(-v)
                ).then_inc(act_sem, 1)
            scalar.wait_ge(dve_sem, 2)
            scalar.dma_start(out=out_d[:, 1], in_=P[:, 0, 1]).then_inc(od_sem, 16)
            scalar.wait_ge(od_sem, 48)

        @block.vector
        def _(vector):
            # stage 1: column-class sums (xp first)
            vector.wait_ge(xa_sem, 16)
            vector.wait_ge(xb_sem, 16)
            vector.tensor_tensor(
                U[:, 1], xp[:, :, :, 2 : 2 + S_COLS], xp[:, :, :, 6 : 6 + S_COLS], Alu.add
            )
            vector.tensor_tensor(
                U[:, 3], xp[:, :, :, 0:S_COLS], xp[:, :, :, 8 : 8 + S_COLS], Alu.add
            )
            # D0[k-1] = xp[rows 4-k] + xp[rows 4+k] — needs only xp; covers
            # the xp1 DMA latency
            for k in range(1, 5):
                vector.tensor_tensor(
                    D0[:, k - 1],
                    xp[:, :, PAD - k : PAD - k + S_ROWS, PAD : PAD + S_COLS],
                    xp[:, :, PAD + k : PAD + k + S_ROWS, PAD : PAD + S_COLS],
                    Alu.add,
                )

            # stage 1b: U1/U3 from the shifted copy xp1
            vector.wait_ge(x1a_sem, 16)
            vector.tensor_tensor(
                U[:, 0], xp1[:, :, :, 2 : 2 + S_COLS], xp1[:, :, :, 4 : 4 + S_COLS], Alu.add
            )
            vector.tensor_tensor(
                U[:, 2], xp1[:, :, :, 0:S_COLS], xp1[:, :, :, 6 : 6 + S_COLS], Alu.add
            )

            # stage 2a: symmetric row-pair sums, batched over the 4 col classes
            for k in range(1, 5):
                vector.tensor_tensor(
                    D[:, k - 1],
                    U[:, :, :, PAD - k : PAD - k + S_ROWS, :],
                    U[:, :, :, PAD + k : PAD + k + S_ROWS, :],
                    Alu.add,
                )
            # stage 2b: ring assembly
            # centers: S[1..4] = U_a[dy=0] + D0[k=a]  (v = 1,4,9,16)
            vector.tensor_tensor(
                S[:, 1:5], U[:, :, :, PAD : PAD + S_ROWS, :], D0[:, :], Alu.add
            )
            def dview(k, a, n, stride):
                # n consecutive D[.] slices stepping by `stride` elements
                src = D[:, k, a]
                return bass.AP(
                    D,
                    src.offset,
                    [list(src.ap[0]), [stride, n]] + [list(x) for x in src.ap[1:]],
                )

            # mixed pairs: S[5..7] = D[1,{2,3,4}] + D[{2,3,4},1]  (v = 5,10,17)
            vector.tensor_tensor(
                S[:, 5:8], dview(0, 1, 3, 768), dview(1, 0, 3, 3072), Alu.add
            )
            # S[8..9] = D[2,{3,4}] + D[{3,4},2]  (v = 13,20)
            vector.tensor_tensor(
                S[:, 8:10], dview(1, 2, 2, 768), dview(2, 1, 2, 3072), Alu.add
            )
            # S[10] = D[3,4] + D[4,3]  (v = 25)
            vector.tensor_tensor(S[:, 10], D[:, 2, 3], D[:, 3, 2], Alu.add)

            # w-gen: Wv[1:15] *= E  (one wide op; host supplied base - r2*B2)
            vector.wait_ge(wv_sem, 16)
            vector.wait_ge(act_sem, 14)
            vector.tensor_tensor(Wv[:, 1:15], Wv[:, 1:15], E[:], Alu.mult)

            def bcast(src, lead=()):
                return bass.AP(
                    src.tensor,
                    src.offset,
                    [list(src.ap[0])]
                    + [list(d) for d in lead]
                    + [[0, C]]
                    + [list(d) for d in src.ap[-2:]],
                )

            # products
            vector.tensor_tensor(
                P[:, 0],
                xp[:, :, PAD : PAD + S_ROWS, PAD : PAD + S_COLS],
                bcast(Wv[:, 0]),
                Alu.mult,
            )
            vector.tensor_tensor(
                P[:, 1:11],
                S[:, 1:11],
                bcast(Wv[:, 1], lead=[[S_ROWS * S_COLS, 10]]),
                Alu.mult,
            )
            # P[11..14] = D[j,j] * Wv[11+j]   (v = 2,8,18,32)
            vector.tensor_tensor(
                P[:, 11:15],
                dview(0, 0, 4, 3840),
                bcast(Wv[:, 11], lead=[[S_ROWS * S_COLS, 4]]),
                Alu.mult,
            )

            # tree-reduce the 15 products (5 ops)
            vector.tensor_tensor(P[:, 0:7], P[:, 0:7], P[:, 7:14], Alu.add)
            vector.tensor_tensor(P[:, 0:3], P[:, 0:3], P[:, 3:6], Alu.add)
            # P[0]+=P[2], P[1]+=P[6] in one op
            p2 = P[:, 2]
            vector.tensor_tensor(
                P[:, 0:2],
                P[:, 0:2],
                bass.AP(
                    P,
                    p2.offset,
                    [list(p2.ap[0]), [4 * 768, 2]] + [list(x) for x in p2.ap[1:]],
                ),
                Alu.add,
            )
            vector.tensor_tensor(P[:, 0], P[:, 0], P[:, 1], Alu.add)
            for c in range(C):
                vector.tensor_tensor(
                    P[:, 0, c], P[:, 0, c], P[:, 14, c], Alu.add
                ).then_inc(dve_sem, 1)

    return nc


_PROGRAM_CACHE = {}


def _get_program():
    if "nc" not in _PROGRAM_CACHE:
        import sys

        if "/opt/trn_rl_repo" not in sys.path:
            sys.path.insert(0, "/opt/trn_rl_repo")
        from concourse import bass, mybir

        nc = bass.Bass()
        _PROGRAM_CACHE["nc"] = _build_program(nc, bass, mybir)
    return _PROGRAM_CACHE["nc"]


def _host_prep(x, foa_xy):
    import ml_dtypes

    bf = ml_dtypes.bfloat16
    xpad = np.pad(x, ((0, 0), (0, 0), (PAD, PAD), (PAD, PAD)), mode="reflect")
    xpad_bf = xpad.astype(bf)
    diag = math.sqrt(H * H + W * W)
    in_maps = []
    for core in range(N_CORES):
        b, half = divmod(core, 2)
        y0 = half * 128
        xph = xpad_bf[b, :, y0 : y0 + 136, :]
        sw = np.lib.stride_tricks.sliding_window_view(xph, (C, IN_R, IN_C))
        XP = np.ascontiguousarray(sw[0, ::S_ROWS, ::S_COLS].reshape(128, C, IN_R, IN_C))
        sw1 = np.lib.stride_tricks.sliding_window_view(xph, (C, IN_R, IN_C1))
        XP1 = np.ascontiguousarray(
            sw1[0, ::S_ROWS, 1::S_COLS][:, :N_BLOCKS].reshape(128, C, IN_R, IN_C1)
        )

        yy, xx = np.meshgrid(
            np.arange(y0, y0 + 128, dtype=np.float64),
            np.arange(W, dtype=np.float64),
            indexing="ij",
        )
        fx, fy = float(foa_xy[b, 0]), float(foa_xy[b, 1])
        dist = np.sqrt((xx - fx) ** 2 + (yy - fy) ** 2)
        dn = dist / diag
        sigma = (1.0 - dn) * SIGMA_MIN + dn * SIGMA_MAX
        inv2s2 = 1.0 / (2.0 * sigma * sigma)
        base = -dist * np.sqrt(sigma) / (math.pi * sigma**4)
        b2 = base * inv2s2

        def tiles(a):
            t = a.reshape(N_STRIPS, S_ROWS, N_BLOCKS, S_COLS)
            return t.transpose(0, 2, 1, 3).reshape(128, S_ROWS, S_COLS)

        wv = np.empty((128, NV, S_ROWS, S_COLS), dtype=bf)
        wv[:, 0] = tiles(base).astype(bf)
        bt, b2t = tiles(base), tiles(b2)
        for i, v in enumerate(V_ORD):
            wv[:, 1 + i] = (bt - v * b2t).astype(bf)
        INV = np.ascontiguousarray(tiles(inv2s2).astype(np.float32))

        in_maps.append(
            {"xp": XP, "xp1": XP1, "wv": np.ascontiguousarray(wv), "inv": INV}
        )
    return in_maps


def _gather(results):
    out = np.empty((B, C, H, W), dtype=np.float32)
    for core in range(N_CORES):
        b, half = divmod(core, 2)
        y0 = half * 128
        o = results[core]["out"].astype(np.float32)
        o = o.reshape(N_STRIPS, N_BLOCKS, C, S_ROWS, S_COLS)
        o = o.transpose(2, 0, 3, 1, 4).reshape(C, 128, W)
        out[b, :, y0 : y0 + 128, :] = o
    return out


def kernel(x, foa_xy, _trace=False, _tmpdir=None):
    import sys

    if "/opt/trn_rl_repo" not in sys.path:
        sys.path.insert(0, "/opt/trn_rl_repo")
    from concourse.bass_utils import run_bass_kernel_spmd

    nc = _get_program()
    in_maps = _host_prep(np.asarray(x), np.asarray(foa_xy))
    kw = {}
    if _trace:
        kw = dict(trace=True, trace_cores=[], tmpdir=_tmpdir)
    res = run_bass_kernel_spmd(nc, in_maps, list(range(N_CORES)), **kw)
    out = _gather(res.results)
    if _trace:
        return out, res
    return out



# revision 6
# speedup vs baseline: 1.0036x; 1.0036x over previous
"""Adaptive per-pixel LoG 9x9 convolution on 8 TRN2 NeuronCores.

out[b,c,y,x] = sum_{dy,dx in [-4,4]} xpad[b,c,y+dy,x+dx] * K(dx^2+dy^2; p)
K depends on the offset only through r2 = dx^2+dy^2 (15 distinct values)
-> exact rank-15 decomposition  out = base * (S_center + sum_v g_v * S_v)
where S_v are fixed ring-sum convolutions (shared shifted adds),
g_v = (1 - t_v) * exp(-t_v), t_v = r2_v * inv2s2, and base/inv2s2 are
smooth per-pixel scalar fields of the focus-of-attention distance.

Sharding: 8 cores = 4 batches x 2 row-halves. Partition p = 16x16 output
tile + 4px halo (24x24 window, 3 channels); all taps are free-dim AP
offsets; host bakes the window layout so DMAs are contiguous.

v2 engine choreography (vs the wv-shipping baseline):
- weight planes are generated ON CHIP: Pool computes the outer product
  T = inv2s2 x (-r2) (14 planes), ACT does one batched E = exp(T), Pool
  fuses G = (T + 1) * E in one scalar_tensor_tensor. Kills the 983KB wv
  DMA (input traffic 1.9MB -> 1.0MB) and the DVE's Wv*=E op.
- xp and xp1 ship as full-128-partition flat halves on the two HWDGE
  queues (partial-partition DMAs hit a 2-of-16 SDMA-engine pathology),
  winv rides SWDGE; DVE starts ~9us earlier.
- DVE does the ring sums in bf16 2x mode (xp1 = 1-col-shifted copy keeps
  taps 4B-aligned), products vs G (zero-stride channel broadcast),
  5-op tree reduce, then out = (acc + center) * base per channel;
  output ships per channel on three queues as each plane completes.
"""

import math

import numpy as np

B, C, H, W = 4, 3, 256, 256
PAD = 4
SIGMA_MIN, SIGMA_MAX = 0.5, 10.0
N_CORES = 8

S_ROWS = 16
S_COLS = 16
N_STRIPS = 8
N_BLOCKS = 16
IN_R = 24
IN_C = 24
IN_C1 = 22

XP_FLAT = C * IN_R * IN_C      # 1728
XP1_FLAT = C * IN_R * IN_C1    # 1584
NW = 528                       # winv blob: base(256) | inv(256) | r2n(14) | pad(2)

R2_VALUES = sorted({dx * dx + dy * dy for dx in range(-4, 5) for dy in range(-4, 5)})
assert len(R2_VALUES) == 15
NV = 15
# ring order: S-resident rings (slots 0..9 of G); D-diagonal rings (10..13)
V_ORD = [1, 4, 9, 16, 5, 10, 17, 13, 20, 25, 2, 8, 18, 32]
NG = 14


def _build_program(nc, bass, mybir):
    bf16 = mybir.dt.bfloat16
    Alu = mybir.AluOpType
    Act = mybir.ActivationFunctionType

    xp_d = nc.declare_dram_parameter("xp", [128, XP_FLAT], bf16, isOutput=False)
    xp1_d = nc.declare_dram_parameter("xp1", [128, XP1_FLAT], bf16, isOutput=False)
    winv_d = nc.declare_dram_parameter("winv", [128, NW], bf16, isOutput=False)
    out_d = nc.declare_dram_parameter("out", [128, C, S_ROWS, S_COLS], bf16, isOutput=True)

    XA, XB = XP_FLAT // 2, XP_FLAT        # xp halves
    X1A, X1B = XP1_FLAT // 2, XP1_FLAT    # xp1 halves

    xa_sem = nc.alloc_semaphore("xa_sem")
    xb_sem = nc.alloc_semaphore("xb_sem")
    x1a_sem = nc.alloc_semaphore("x1a_sem")
    x1b_sem = nc.alloc_semaphore("x1b_sem")
    w_sem = nc.alloc_semaphore("w_sem")
    t_sem = nc.alloc_semaphore("t_sem")
    e_sem = nc.alloc_semaphore("e_sem")
    g_sem = nc.alloc_semaphore("g_sem")
    dve_sem = nc.alloc_semaphore("dve_sem")
    od_sem = nc.alloc_semaphore("od_sem")
    od2_sem = nc.alloc_semaphore("od2_sem")
    xp = nc.alloc_sbuf_tensor("s_xp", [128, C, IN_R, IN_C], bf16)
    xp1 = nc.alloc_sbuf_tensor("s_xp1", [128, C, IN_R, IN_C1], bf16)
    winv = nc.alloc_sbuf_tensor("s_winv", [128, NW], bf16)
    T = nc.alloc_sbuf_tensor("T", [128, NG, S_ROWS * S_COLS], bf16)
    E = nc.alloc_sbuf_tensor("E", [128, NG, S_ROWS * S_COLS], bf16)
    G = nc.alloc_sbuf_tensor("G", [128, NG, S_ROWS * S_COLS], bf16)
    U = nc.alloc_sbuf_tensor("U", [128, 4, C, IN_R, S_COLS], bf16)
    D = nc.alloc_sbuf_tensor("D", [128, 4, 4, C, S_ROWS, S_COLS], bf16)
    D0 = nc.alloc_sbuf_tensor("D0", [128, 4, C, S_ROWS, S_COLS], bf16)
    S = nc.alloc_sbuf_tensor("S", [128, 11, C, S_ROWS, S_COLS], bf16)
    P = nc.alloc_sbuf_tensor("P", [128, NG, C, S_ROWS, S_COLS], bf16)
    O = nc.alloc_sbuf_tensor("O", [128, C, S_ROWS, S_COLS], bf16)

    with nc.Block() as block:
        def flat(t, lo, hi):
            # flat per-partition [lo:hi) element view of an SBUF tensor
            a = t[:]
            return bass.AP(t, lo, [list(a.ap[0]), [1, hi - lo]])

        @block.sync
        def _(sync):
            sync.dma_start(out=flat(xp, 0, XA), in_=xp_d[:, 0:XA]).then_inc(xa_sem, 16)
            sync.dma_start(out=flat(xp1, 0, X1A), in_=xp1_d[:, 0:X1A]).then_inc(
                x1a_sem, 16
            )
            sync.wait_ge(dve_sem, 1)
            sync.dma_start(out=out_d[:, 0], in_=O[:, 0]).then_inc(od_sem, 16)
            sync.wait_ge(od_sem, 32)
            sync.wait_ge(od2_sem, 16)

        @block.gpsimd
        def _(gpsimd):
            gpsimd.dma_start(out=winv[:], in_=winv_d[:]).then_inc(w_sem, 16)
            gpsimd.wait_ge(w_sem, 16)
            # T[v, p] = inv2s2[p] * (-r2_v)   (outer product via broadcasts)
            inv_b = bass.AP(winv, 256, [list(winv[:].ap[0]), [0, NG], [1, 256]])
            r2n_b = bass.AP(winv, 512, [list(winv[:].ap[0]), [1, NG], [0, 256]])
            gpsimd.tensor_tensor(T[:], inv_b, r2n_b, Alu.mult).then_inc(t_sem, 1)
            gpsimd.wait_ge(dve_sem, 3)
            gpsimd.dma_start(out=out_d[:, 2], in_=O[:, 2]).then_inc(od2_sem, 16)
            gpsimd.wait_ge(od_sem, 32)
            gpsimd.wait_ge(od2_sem, 16)

        @block.scalar
        def _(scalar):
            scalar.dma_start(out=flat(xp, XA, XB), in_=xp_d[:, XA:XB]).then_inc(
                xb_sem, 16
            )
            scalar.dma_start(out=flat(xp1, X1A, X1B), in_=xp1_d[:, X1A:X1B]).then_inc(
                x1b_sem, 16
            )
            scalar.wait_ge(t_sem, 1)
            scalar.activation(E[:], T[:], Act.Exp, bias=0.0, scale=1.0).then_inc(
                e_sem, 1
            )
            scalar.wait_ge(dve_sem, 2)
            scalar.dma_start(out=out_d[:, 1], in_=O[:, 1]).then_inc(od_sem, 16)
            scalar.wait_ge(od_sem, 32)
            scalar.wait_ge(od2_sem, 16)

        @block.vector
        def _(vector):
            # stage 1: column-class sums from xp
            vector.wait_ge(xa_sem, 16)
            vector.wait_ge(xb_sem, 16)
            vector.tensor_tensor(
                U[:, 1], xp[:, :, :, 2 : 2 + S_COLS], xp[:, :, :, 6 : 6 + S_COLS], Alu.add
            )
            vector.tensor_tensor(
                U[:, 3], xp[:, :, :, 0:S_COLS], xp[:, :, :, 8 : 8 + S_COLS], Alu.add
            )
            # D0[k-1] = xp[rows 4-k] + xp[rows 4+k] (center col class)
            for k in range(1, 5):
                vector.tensor_tensor(
                    D0[:, k - 1],
                    xp[:, :, PAD - k : PAD - k + S_ROWS, PAD : PAD + S_COLS],
                    xp[:, :, PAD + k : PAD + k + S_ROWS, PAD : PAD + S_COLS],
                    Alu.add,
                )

            # stage 1b: U0/U2 from the shifted copy xp1
            vector.wait_ge(x1a_sem, 16)
            vector.wait_ge(x1b_sem, 16)
            vector.tensor_tensor(
                U[:, 0], xp1[:, :, :, 2 : 2 + S_COLS], xp1[:, :, :, 4 : 4 + S_COLS], Alu.add
            )
            vector.tensor_tensor(
                U[:, 2], xp1[:, :, :, 0:S_COLS], xp1[:, :, :, 6 : 6 + S_COLS], Alu.add
            )

            # stage 2a: symmetric row-pair sums, batched over the 4 col classes
            for k in range(1, 5):
                vector.tensor_tensor(
                    D[:, k - 1],
                    U[:, :, :, PAD - k : PAD - k + S_ROWS, :],
                    U[:, :, :, PAD + k : PAD + k + S_ROWS, :],
                    Alu.add,
                )
            # stage 2b: ring assembly
            # centers: S[1..4] = U_a[dy=0] + D0[k=a]  (v = 1,4,9,16)
            vector.tensor_tensor(
                S[:, 1:5], U[:, :, :, PAD : PAD + S_ROWS, :], D0[:, :], Alu.add
            )

            def dview(k, a, n, stride):
                # n consecutive D[.] slices stepping by `stride` elements
                src = D[:, k, a]
                return bass.AP(
                    D,
                    src.offset,
                    [list(src.ap[0]), [stride, n]] + [list(x) for x in src.ap[1:]],
                )

            # mixed pairs: S[5..7] = D[1,{2,3,4}] + D[{2,3,4},1]  (v = 5,10,17)
            vector.tensor_tensor(
                S[:, 5:8], dview(0, 1, 3, 768), dview(1, 0, 3, 3072), Alu.add
            )
            # S[8..9] = D[2,{3,4}] + D[{3,4},2]  (v = 13,20)
            vector.tensor_tensor(
                S[:, 8:10], dview(1, 2, 2, 768), dview(2, 1, 2, 3072), Alu.add
            )
            # S[10] = D[3,4] + D[4,3]  (v = 25)
            vector.tensor_tensor(S[:, 10], D[:, 2, 3], D[:, 3, 2], Alu.add)

            def gb(i, n):
                # G planes [i:i+n) broadcast over the channel dim
                a = G[:]
                return bass.AP(
                    G,
                    i * 256,
                    [list(a.ap[0]), [256, n], [0, C], [S_COLS, S_ROWS], [1, S_COLS]],
                )

            # G = (T + 1) * E = (1 - t) * exp(-t), fused on the DVE
            vector.wait_ge(e_sem, 1)
            vector.scalar_tensor_tensor(G[:], T[:], 1.0, E[:], Alu.add, Alu.mult)

            # products: P[0..9] = S[1..10] * g, P[10..13] = D[j,j] * g
            vector.tensor_tensor(P[:, 0:10], S[:, 1:11], gb(0, 10), Alu.mult)
            vector.tensor_tensor(P[:, 10:14], dview(0, 0, 4, 3840), gb(10, 4), Alu.mult)

            # tree-reduce the 14 products (4 ops)
            vector.tensor_tensor(P[:, 0:7], P[:, 0:7], P[:, 7:14], Alu.add)
            vector.tensor_tensor(P[:, 0:3], P[:, 0:3], P[:, 3:6], Alu.add)
            # P[0]+=P[2], P[1]+=P[6] in one op
            p2 = P[:, 2]
            vector.tensor_tensor(
                P[:, 0:2],
                P[:, 0:2],
                bass.AP(
                    P,
                    p2.offset,
                    [list(p2.ap[0]), [4 * 768, 2]] + [list(x) for x in p2.ap[1:]],
                ),
                Alu.add,
            )
            vector.tensor_tensor(P[:, 0], P[:, 0], P[:, 1], Alu.add)
            # acc += center pixel (ring r2=0 has weight base*1)
            vector.tensor_tensor(
                P[:, 1],
                P[:, 0],
                xp[:, :, PAD : PAD + S_ROWS, PAD : PAD + S_COLS],
                Alu.add,
            )
            # out[c] = acc[c] * base, per channel so planes ship as they finish
            base_b = bass.AP(
                winv, 0, [list(winv[:].ap[0]), [S_COLS, S_ROWS], [1, S_COLS]]
            )
            for c in range(C):
                vector.tensor_tensor(O[:, c], P[:, 1, c], base_b, Alu.mult).then_inc(
                    dve_sem, 1
                )

    return nc


_PROGRAM_CACHE = {}


def _get_program():
    if "nc" not in _PROGRAM_CACHE:
        import sys

        if "/opt/trn_rl_repo" not in sys.path:
            sys.path.insert(0, "/opt/trn_rl_repo")
        from concourse import bass, mybir

        nc = bass.Bass()
        _PROGRAM_CACHE["nc"] = _build_program(nc, bass, mybir)
    return _PROGRAM_CACHE["nc"]


def _host_prep(x, foa_xy):
    import ml_dtypes

    bf = ml_dtypes.bfloat16
    xpad = np.pad(x, ((0, 0), (0, 0), (PAD, PAD), (PAD, PAD)), mode="reflect")
    xpad_bf = xpad.astype(bf)
    diag = math.sqrt(H * H + W * W)
    in_maps = []
    for core in range(N_CORES):
        b, half = divmod(core, 2)
        y0 = half * 128
        xph = xpad_bf[b, :, y0 : y0 + 136, :]
        sw = np.lib.stride_tricks.sliding_window_view(xph, (C, IN_R, IN_C))
        XP = np.ascontiguousarray(
            sw[0, ::S_ROWS, ::S_COLS].reshape(128, XP_FLAT)
        )
        sw1 = np.lib.stride_tricks.sliding_window_view(xph, (C, IN_R, IN_C1))
        XP1 = np.ascontiguousarray(
            sw1[0, ::S_ROWS, 1::S_COLS][:, :N_BLOCKS].reshape(128, XP1_FLAT)
        )

        yy, xx = np.meshgrid(
            np.arange(y0, y0 + 128, dtype=np.float64),
            np.arange(W, dtype=np.float64),
            indexing="ij",
        )
        fx, fy = float(foa_xy[b, 0]), float(foa_xy[b, 1])
        dist = np.sqrt((xx - fx) ** 2 + (yy - fy) ** 2)
        dn = dist / diag
        sigma = (1.0 - dn) * SIGMA_MIN + dn * SIGMA_MAX
        inv2s2 = 1.0 / (2.0 * sigma * sigma)
        base = -dist * np.sqrt(sigma) / (math.pi * sigma**4)

        def tiles(a):
            t = a.reshape(N_STRIPS, S_ROWS, N_BLOCKS, S_COLS)
            return t.transpose(0, 2, 1, 3).reshape(128, S_ROWS * S_COLS)

        winv = np.empty((128, NW), dtype=bf)
        winv[:, 0:256] = tiles(base).astype(bf)
        winv[:, 256:512] = tiles(inv2s2).astype(bf)
        winv[:, 512 : 512 + NG] = np.asarray(
            [-v for v in V_ORD], dtype=np.float64
        ).astype(bf)[None, :]
        winv[:, 512 + NG :] = 0

        in_maps.append({"xp": XP, "xp1": XP1, "winv": np.ascontiguousarray(winv)})
    return in_maps


def _gather(results):
    out = np.empty((B, C, H, W), dtype=np.float32)
    for core in range(N_CORES):
        b, half = divmod(core, 2)
        y0 = half * 128
        o = results[core]["out"].astype(np.float32)
        o = o.reshape(N_STRIPS, N_BLOCKS, C, S_ROWS, S_COLS)
        o = o.transpose(2, 0, 3, 1, 4).reshape(C, 128, W)
        out[b, :, y0 : y0 + 128, :] = o
    return out


def kernel(x, foa_xy, _trace=False, _tmpdir=None):
    import sys

    if "/opt/trn_rl_repo" not in sys.path:
        sys.path.insert(0, "/opt/trn_rl_repo")
    from concourse.bass_utils import run_bass_kernel_spmd

    nc = _get_program()
    in_maps = _host_prep(np.asarray(x), np.asarray(foa_xy))
    kw = {}
    if _trace:
        kw = dict(trace=True, trace_cores=[], tmpdir=_tmpdir)
    res = run_bass_kernel_spmd(nc, in_maps, list(range(N_CORES)), **kw)
    out = _gather(res.results)
    if _trace:
        return out, res
    return out


# revision 8
# speedup vs baseline: 1.1937x; 1.1895x over previous
"""Adaptive per-pixel LoG 9x9 convolution on 8 TRN2 NeuronCores.

out[b,c,y,x] = sum_{dy,dx in [-4,4]} xpad[b,c,y+dy,x+dx] * K(dx^2+dy^2; p)
K depends on the offset only through r2 = dx^2+dy^2 (15 distinct values)
-> exact rank-15 decomposition  out = base * (S_center + sum_v g_v * S_v)
where S_v are fixed ring-sum convolutions (shared shifted adds),
g_v = (1 - t_v) * exp(-t_v), t_v = r2_v * inv2s2, and base/inv2s2 are
smooth per-pixel scalar fields of the focus-of-attention distance.

Sharding: 8 cores = 4 batches x 2 row-halves. Partition p = 16x16 output
tile + 4px halo (24x24 window, 3 channels); all taps are free-dim AP
offsets; host bakes the window layout so DMAs are contiguous.

v2 engine choreography (vs the wv-shipping baseline):
- weight planes are generated ON CHIP: Pool computes the outer product
  T = inv2s2 x (-r2) (14 planes), ACT does one batched E = exp(T), Pool
  fuses G = (T + 1) * E in one scalar_tensor_tensor. Kills the 983KB wv
  DMA (input traffic 1.9MB -> 1.0MB) and the DVE's Wv*=E op.
- xp and xp1 ship as full-128-partition flat halves on the two HWDGE
  queues (partial-partition DMAs hit a 2-of-16 SDMA-engine pathology),
  winv rides SWDGE; DVE starts ~9us earlier.
- DVE does the ring sums in bf16 2x mode (xp1 = 1-col-shifted copy keeps
  taps 4B-aligned), products vs G (zero-stride channel broadcast),
  5-op tree reduce, then out = (acc + center) * base per channel;
  output ships per channel on three queues as each plane completes.
"""

import math

import numpy as np

B, C, H, W = 4, 3, 256, 256
PAD = 4
SIGMA_MIN, SIGMA_MAX = 0.5, 10.0
N_CORES = 8

S_ROWS = 16
S_COLS = 16
N_STRIPS = 8
N_BLOCKS = 16
IN_R = 24
IN_C = 24
IN_C1 = 22

XP_FLAT = C * IN_R * IN_C      # 1728
XP1_FLAT = C * IN_R * IN_C1    # 1584
NW = 528                       # winv blob: base(256) | inv(256) | r2n(14) | pad(2)

R2_VALUES = sorted({dx * dx + dy * dy for dx in range(-4, 5) for dy in range(-4, 5)})
assert len(R2_VALUES) == 15
NV = 15
# ring order: S-resident rings (slots 0..9 of G); D-diagonal rings (10..13)
V_ORD = [1, 4, 9, 16, 5, 10, 17, 13, 20, 25, 2, 8, 18, 32]
NG = 14


def _build_program(nc, bass, mybir):
    bf16 = mybir.dt.bfloat16
    Alu = mybir.AluOpType
    Act = mybir.ActivationFunctionType

    xp_d = nc.declare_dram_parameter("xp", [128, XP_FLAT], bf16, isOutput=False)
    xp1_d = nc.declare_dram_parameter("xp1", [128, XP1_FLAT], bf16, isOutput=False)
    winv_d = nc.declare_dram_parameter("winv", [128, NW], bf16, isOutput=False)
    out_d = nc.declare_dram_parameter("out", [128, C, S_ROWS, S_COLS], bf16, isOutput=True)

    XA, XB = XP_FLAT // 2, XP_FLAT        # xp halves
    X1A, X1B = XP1_FLAT // 2, XP1_FLAT    # xp1 halves

    xa_sem = nc.alloc_semaphore("xa_sem")
    xb_sem = nc.alloc_semaphore("xb_sem")
    x1a_sem = nc.alloc_semaphore("x1a_sem")
    x1b_sem = nc.alloc_semaphore("x1b_sem")
    w_sem = nc.alloc_semaphore("w_sem")
    e_sem = nc.alloc_semaphore("e_sem")
    dve_sem = nc.alloc_semaphore("dve_sem")
    od_sem = nc.alloc_semaphore("od_sem")
    xp = nc.alloc_sbuf_tensor("s_xp", [128, C, IN_R, IN_C], bf16)
    xp1 = nc.alloc_sbuf_tensor("s_xp1", [128, C, IN_R, IN_C1], bf16)
    winv = nc.alloc_sbuf_tensor("s_winv", [128, NW], bf16)
    F = nc.alloc_sbuf_tensor("F", [128, NG, S_ROWS * S_COLS], bf16)
    E = nc.alloc_sbuf_tensor("E", [128, NG, S_ROWS * S_COLS], bf16)
    G = nc.alloc_sbuf_tensor("G", [128, NG, S_ROWS * S_COLS], bf16)
    U = nc.alloc_sbuf_tensor("U", [128, 4, C, IN_R, S_COLS], bf16)
    D = nc.alloc_sbuf_tensor("D", [128, 4, 4, C, S_ROWS, S_COLS], bf16)
    D0 = nc.alloc_sbuf_tensor("D0", [128, 4, C, S_ROWS, S_COLS], bf16)
    S = nc.alloc_sbuf_tensor("S", [128, 11, C, S_ROWS, S_COLS], bf16)
    P = nc.alloc_sbuf_tensor("P", [128, NG, C, S_ROWS, S_COLS], bf16)
    O = nc.alloc_sbuf_tensor("O", [128, C, S_ROWS, S_COLS], bf16)

    with nc.Block() as block:
        def flat(t, lo, hi):
            # flat per-partition [lo:hi) element view of an SBUF tensor
            a = t[:]
            return bass.AP(t, lo, [list(a.ap[0]), [1, hi - lo]])

        @block.sync
        def _(sync):
            sync.dma_start(out=flat(xp, 0, XA), in_=xp_d[:, 0:XA]).then_inc(xa_sem, 16)
            sync.dma_start(out=flat(xp1, 0, X1A), in_=xp1_d[:, 0:X1A]).then_inc(
                x1a_sem, 16
            )
            sync.wait_ge(dve_sem, 1)
            sync.dma_start(out=out_d[:, 0], in_=O[:, 0]).then_inc(od_sem, 16)
            sync.wait_ge(dve_sem, 3)
            sync.dma_start(out=out_d[:, 2], in_=O[:, 2]).then_inc(od_sem, 16)
            sync.wait_ge(od_sem, 48)

        @block.gpsimd
        def _(gpsimd):
            gpsimd.dma_start(out=winv[:], in_=winv_d[:]).then_inc(w_sem, 16)
            gpsimd.wait_ge(od_sem, 48)

        @block.scalar
        def _(scalar):
            scalar.dma_start(out=flat(xp, XA, XB), in_=xp_d[:, XA:XB]).then_inc(
                xb_sem, 16
            )
            scalar.dma_start(out=flat(xp1, X1A, X1B), in_=xp1_d[:, X1A:X1B]).then_inc(
                x1b_sem, 16
            )
            scalar.wait_ge(w_sem, 16)
            inv_ap = bass.AP(winv, 256, [list(winv[:].ap[0]), [1, 256]])
            # F[v] = 1 - v*inv2s2 ; E[v] = exp(-v*inv2s2)
            for i, v in enumerate(V_ORD):
                scalar.activation(
                    F[:, i], inv_ap, Act.Identity, bias=1.0, scale=float(-v)
                )
            for i, v in enumerate(V_ORD):
                act = scalar.activation(
                    E[:, i], inv_ap, Act.Exp, bias=0.0, scale=float(-v)
                )
            act.then_inc(e_sem, 1)
            scalar.wait_ge(dve_sem, 2)
            scalar.dma_start(out=out_d[:, 1], in_=O[:, 1]).then_inc(od_sem, 16)
            scalar.wait_ge(od_sem, 48)

        @block.vector
        def _(vector):
            # stage 1: column-class sums from xp
            vector.wait_ge(xa_sem, 16)
            vector.wait_ge(xb_sem, 16)
            pU = list(U[:].ap[0])
            vector.tensor_tensor(
                U[:, 1], xp[:, :, :, 2 : 2 + S_COLS], xp[:, :, :, 6 : 6 + S_COLS], Alu.add
            )
            vector.tensor_tensor(
                U[:, 3], xp[:, :, :, 0:S_COLS], xp[:, :, :, 8 : 8 + S_COLS], Alu.add
            )
            # D0[k-1] = xp[rows 4-k] + xp[rows 4+k] (center col class)
            for k in range(1, 5):
                vector.tensor_tensor(
                    D0[:, k - 1],
                    xp[:, :, PAD - k : PAD - k + S_ROWS, PAD : PAD + S_COLS],
                    xp[:, :, PAD + k : PAD + k + S_ROWS, PAD : PAD + S_COLS],
                    Alu.add,
                )

            # stage 1b: U0/U2 from the shifted copy xp1
            vector.wait_ge(x1a_sem, 16)
            vector.wait_ge(x1b_sem, 16)
            vector.tensor_tensor(
                U[:, 0], xp1[:, :, :, 2 : 2 + S_COLS], xp1[:, :, :, 4 : 4 + S_COLS], Alu.add
            )
            vector.tensor_tensor(
                U[:, 2], xp1[:, :, :, 0:S_COLS], xp1[:, :, :, 6 : 6 + S_COLS], Alu.add
            )

            # stage 2a: symmetric row-pair sums, all k and col classes in one op
            vector.tensor_tensor(
                D[:],
                bass.AP(U, 48, [pU, [-16, 4], [1152, 4], [384, 3], [16, S_ROWS], [1, S_COLS]]),
                bass.AP(U, 80, [pU, [16, 4], [1152, 4], [384, 3], [16, S_ROWS], [1, S_COLS]]),
                Alu.add,
            )
            # stage 2b: ring assembly
            # centers: S[1..4] = U_a[dy=0] + D0[k=a]  (v = 1,4,9,16)
            vector.tensor_tensor(
                S[:, 1:5], U[:, :, :, PAD : PAD + S_ROWS, :], D0[:, :], Alu.add
            )

            def dview(k, a, n, stride):
                # n consecutive D[.] slices stepping by `stride` elements
                src = D[:, k, a]
                return bass.AP(
                    D,
                    src.offset,
                    [list(src.ap[0]), [stride, n]] + [list(x) for x in src.ap[1:]],
                )

            # mixed pairs: S[5..7] = D[1,{2,3,4}] + D[{2,3,4},1]  (v = 5,10,17)
            vector.tensor_tensor(
                S[:, 5:8], dview(0, 1, 3, 768), dview(1, 0, 3, 3072), Alu.add
            )
            # S[8..9] = D[2,{3,4}] + D[{3,4},2]  (v = 13,20)
            vector.tensor_tensor(
                S[:, 8:10], dview(1, 2, 2, 768), dview(2, 1, 2, 3072), Alu.add
            )
            # S[10] = D[3,4] + D[4,3]  (v = 25)
            vector.tensor_tensor(S[:, 10], D[:, 2, 3], D[:, 3, 2], Alu.add)

            def gb(i, n):
                # G planes [i:i+n) broadcast over the channel dim
                a = G[:]
                return bass.AP(
                    G,
                    i * 256,
                    [list(a.ap[0]), [256, n], [0, C], [S_COLS, S_ROWS], [1, S_COLS]],
                )

            # G = F * E = (1 - t) * exp(-t)
            vector.wait_ge(e_sem, 1)
            vector.tensor_tensor(G[:], F[:], E[:], Alu.mult)

            # products: P[0..9] = S[1..10] * g, P[10..13] = D[j,j] * g
            vector.tensor_tensor(P[:, 0:10], S[:, 1:11], gb(0, 10), Alu.mult)
            vector.tensor_tensor(P[:, 10:14], dview(0, 0, 4, 3840), gb(10, 4), Alu.mult)

            # tree-reduce the 14 products (4 ops)
            vector.tensor_tensor(P[:, 0:7], P[:, 0:7], P[:, 7:14], Alu.add)
            vector.tensor_tensor(P[:, 0:3], P[:, 0:3], P[:, 3:6], Alu.add)
            # P[0]+=P[2], P[1]+=P[6] in one op
            p2 = P[:, 2]
            vector.tensor_tensor(
                P[:, 0:2],
                P[:, 0:2],
                bass.AP(
                    P,
                    p2.offset,
                    [list(p2.ap[0]), [4 * 768, 2]] + [list(x) for x in p2.ap[1:]],
                ),
                Alu.add,
            )
            vector.tensor_tensor(P[:, 0], P[:, 0], P[:, 1], Alu.add)
            # acc += center pixel (ring r2=0 has weight base*1)
            vector.tensor_tensor(
                P[:, 1],
                P[:, 0],
                xp[:, :, PAD : PAD + S_ROWS, PAD : PAD + S_COLS],
                Alu.add,
            )
            # out[c] = acc[c] * base, per channel so planes ship as they finish
            base_b = bass.AP(
                winv, 0, [list(winv[:].ap[0]), [S_COLS, S_ROWS], [1, S_COLS]]
            )
            for c in range(C):
                vector.tensor_tensor(O[:, c], P[:, 1, c], base_b, Alu.mult).then_inc(
                    dve_sem, 1
                )

    return nc


_PROGRAM_CACHE = {}


def _get_program():
    if "nc" not in _PROGRAM_CACHE:
        import sys

        if "/opt/trn_rl_repo" not in sys.path:
            sys.path.insert(0, "/opt/trn_rl_repo")
        from concourse import bass, mybir

        nc = bass.Bass()
        _PROGRAM_CACHE["nc"] = _build_program(nc, bass, mybir)
    return _PROGRAM_CACHE["nc"]


def _host_prep(x, foa_xy):
    import ml_dtypes

    bf = ml_dtypes.bfloat16
    xpad = np.pad(x, ((0, 0), (0, 0), (PAD, PAD), (PAD, PAD)), mode="reflect")
    xpad_bf = xpad.astype(bf)
    diag = math.sqrt(H * H + W * W)
    in_maps = []
    for core in range(N_CORES):
        b, half = divmod(core, 2)
        y0 = half * 128
        xph = xpad_bf[b, :, y0 : y0 + 136, :]
        sw = np.lib.stride_tricks.sliding_window_view(xph, (C, IN_R, IN_C))
        XP = np.ascontiguousarray(
            sw[0, ::S_ROWS, ::S_COLS].reshape(128, XP_FLAT)
        )
        sw1 = np.lib.stride_tricks.sliding_window_view(xph, (C, IN_R, IN_C1))
        XP1 = np.ascontiguousarray(
            sw1[0, ::S_ROWS, 1::S_COLS][:, :N_BLOCKS].reshape(128, XP1_FLAT)
        )

        yy, xx = np.meshgrid(
            np.arange(y0, y0 + 128, dtype=np.float64),
            np.arange(W, dtype=np.float64),
            indexing="ij",
        )
        fx, fy = float(foa_xy[b, 0]), float(foa_xy[b, 1])
        dist = np.sqrt((xx - fx) ** 2 + (yy - fy) ** 2)
        dn = dist / diag
        sigma = (1.0 - dn) * SIGMA_MIN + dn * SIGMA_MAX
        inv2s2 = 1.0 / (2.0 * sigma * sigma)
        base = -dist * np.sqrt(sigma) / (math.pi * sigma**4)

        def tiles(a):
            t = a.reshape(N_STRIPS, S_ROWS, N_BLOCKS, S_COLS)
            return t.transpose(0, 2, 1, 3).reshape(128, S_ROWS * S_COLS)

        winv = np.empty((128, NW), dtype=bf)
        winv[:, 0:256] = tiles(base).astype(bf)
        winv[:, 256:512] = tiles(inv2s2).astype(bf)
        winv[:, 512 : 512 + NG] = np.asarray(
            [-v for v in V_ORD], dtype=np.float64
        ).astype(bf)[None, :]
        winv[:, 512 + NG :] = 0

        in_maps.append({"xp": XP, "xp1": XP1, "winv": np.ascontiguousarray(winv)})
    return in_maps


def _gather(results):
    out = np.empty((B, C, H, W), dtype=np.float32)
    for core in range(N_CORES):
        b, half = divmod(core, 2)
        y0 = half * 128
        o = results[core]["out"].astype(np.float32)
        o = o.reshape(N_STRIPS, N_BLOCKS, C, S_ROWS, S_COLS)
        o = o.transpose(2, 0, 3, 1, 4).reshape(C, 128, W)
        out[b, :, y0 : y0 + 128, :] = o
    return out


def kernel(x, foa_xy, _trace=False, _tmpdir=None):
    import sys

    if "/opt/trn_rl_repo" not in sys.path:
        sys.path.insert(0, "/opt/trn_rl_repo")
    from concourse.bass_utils import run_bass_kernel_spmd

    nc = _get_program()
    in_maps = _host_prep(np.asarray(x), np.asarray(foa_xy))
    kw = {}
    if _trace:
        kw = dict(trace=True, trace_cores=[], tmpdir=_tmpdir)
    res = run_bass_kernel_spmd(nc, in_maps, list(range(N_CORES)), **kw)
    out = _gather(res.results)
    if _trace:
        return out, res
    return out
